# revision 20
# baseline (speedup 1.0000x reference)
"""Chamfer (AutoEncoder) loss on 8 Trainium2 NeuronCores.

Problem: predictions [16, 2048, 3], targets [16, 2048, 3] (float32).
loss = sum_b [ sum_i min_j ||x_bi - y_bj||^2 + sum_j min_i ||x_bi - y_bj||^2 ]

Strategy (v3: hybrid dual-fold / paired-fold)
---------------------------------------------
Data-parallel over the batch: 2 batches per core. The DVE is the only
engine that can compute max (HW probes: ACT accumulates sums only,
GPSIMD rejects tensor-tensor ops, no DVE 2x perf modes engage), so
every distance-matrix element must stream through it; the whole design
minimizes DVE instruction count and kernel tail.

- Batch 0 (job 0, "single-P dual-fold"): each [128, 2048] strip of -P
  is drained by a custom DVE op that in one pass writes the running
  column accumulator (out = max(strip, acc)) AND folds the strip's row
  maxes (accum_out = fold(Src0); hand-edited uop program). This touches
  each element exactly once (1 fresh el/lane/cycle) and computes BOTH
  reduction directions from one set of matmuls. The final column
  accumulator is DMA'd to the host (hides under batch 1's compute).

- Batch 1 (jobs 1-2, "two-direction paired"): -P and -P^T are computed
  separately; each strip is row-folded by a paired DVE read (PSUM lower
  half + ACT-copied upper half = 2 fresh els/lane/cycle). Same total
  DVE cycles as dual-fold, but needs NO column tail - the kernel ends
  at the last DVE instruction instead of a transpose+reduce chain.

The distance matrix comes from K-stacked bf16 matmuls (augmentation
trick, hi/lo split, K=15; PE time scales with output columns, not K)
with PE row-group rotation at partition offsets 0/32/64.
"""

import ml_dtypes
import numpy as np

import concourse.dve_ops as dve_ops
import concourse.mybir as mybir
import concourse.tile as tile
from concourse import bacc
from concourse.bass_utils import run_bass_kernel_spmd
from concourse.dve_ops import DveOp
from concourse.dve_spec import Spec, Src0, Src1, _has_src1, lower, maxx, minn
from concourse.dve_uop import DelayInp, DveOpSpec, OutPath, OutSel


def _fold_free(a):
    return np.max(a.astype(np.float32), axis=tuple(range(1, a.ndim))).reshape(
        a.shape[0], 1
    )


def _register_op(name, spec, edit=None):
    """Register a custom DVE op; optionally hand-edit the lowered uops.

    Edited programs are injected into dve_ops._COMPILE_CACHE so both the
    per-NEFF table generator and the instruction emitter use them.
    """
    for existing in dve_ops.OPS:
        if existing.name == name:
            return existing
    row = dve_ops._CUSTOM_DVE_ROW_BASE + len(dve_ops.OPS)
    shas = {}
    compiled = {}
    for ver in ("v3", "v4"):
        try:
            uops = lower(spec, ver=ver)
        except Exception:
            continue
        if edit is not None:
            uops = edit(uops, ver)
        s = DveOpSpec(name=name, opcode=row, uops=uops, rd1_en=_has_src1(spec))
        s.validate(ver)
        shas[ver] = s.sha(ver)
        compiled[ver] = s
    op = DveOp(name, spec, subdim=False, uops_sha=shas)
    dve_ops.OPS.append(op)
    dve_ops._SUB_OPCODE_FOR_NAME[op.name] = row
    dve_ops.CUSTOM_DVE_SPECS[op.name] = op.spec
    for ver, s in compiled.items():
        dve_ops._COMPILE_CACHE[(name, ver)] = s
    assert max(dve_ops._SUB_OPCODE_FOR_NAME.values()) < 0x20
    return op


def _register_max2():
    """out = max(Src0, Src1); accum_out = max-fold(out). (v1 paired fold)"""
    spec = Spec(
        body=maxx(Src0, Src1),
        accum=maxx,
        reference=lambda in0, in1, s0, s1, imm2: (
            np.maximum(in0.astype(np.float32), in1.astype(np.float32)),
            _fold_free(np.maximum(in0.astype(np.float32), in1.astype(np.float32))),
        ),
    )
    return _register_op("MAX2_REDUCE_ANT", spec)


def _register_maxpair_fold():
    """out = max(Src0, Src1); accum_out = max-fold(Src0) (dual-fold).

    Body min(Src0, max(Src0, Src1)) == Src0 makes lower() fold Src0;
    the hand edit reroutes `out` to the dp[0] pair max via delay lane 3.
    """

    def edit(uops, ver):
        assert len(uops) == 2, f"expected seed+steady, got {len(uops)}"
        seed, steady = uops
        assert steady.require_inp0 == 1, "uop order changed"
        for u in uops:
            for dp in u.datapath_config:
                dp.delay[3] = DelayInp.PREV_DELAY
                dp.delay_enable[3] = 1
        steady.datapath_config[1].delay[3] = DelayInp.PREV_ALU_OUT
        steady.out[OutPath.WR0_LO] = OutSel.DELAY_3
        return uops

    spec = Spec(
        body=minn(Src0, maxx(Src0, Src1)),
        accum=maxx,
        reference=lambda in0, in1, s0, s1, imm2: (
            np.maximum(in0.astype(np.float32), in1.astype(np.float32)),
            _fold_free(in0),
        ),
    )
    return _register_op("MAXPAIR_FOLD0_ANT", spec, edit)


def _register_copy_fold():
    """out = Src0 (accumulator init); accum_out = max-fold(Src0)."""
    spec = Spec(
        body=Src0,
        accum=maxx,
        reference=lambda in0, in1, s0, s1, imm2: (
            in0.astype(np.float32),
            _fold_free(in0),
        ),
    )
    return _register_op("COPY_FOLD0_ANT", spec)


MAX2_REDUCE = _register_max2()
MAXPAIR_FOLD = _register_maxpair_fold()
COPY_FOLD = _register_copy_fold()

B, N, M, D = 16, 2048, 2048, 3
N_CORES = 8
ROW_TILES = N // 128  # 16
COL_CHUNK = 512
KCAT = 15  # [hi; hi; lo] x [hi; lo; hi]
NJOBS = 3  # job0: batch0 single-P; jobs 1-2: batch1 dir0/dir1

_F32 = mybir.dt.float32
_BF16 = mybir.dt.bfloat16
_NP_BF16 = ml_dtypes.bfloat16

_cached_nc = None


def _build_nc():
    nc = bacc.Bacc("TRN2", target_bir_lowering=False, debug=False)
    # lhs+rhs packed per replica: one DMA per replica loads both
    ops = nc.dram_tensor("ops", [NJOBS, 3, KCAT, 2 * N], _BF16, kind="ExternalInput")
    # job0 lo/hi half row folds (host maxes the pair)
    rowm = nc.dram_tensor("rowm", [128, 2 * ROW_TILES], _F32, kind="ExternalOutput")
    # job0 final column accumulator, folded over rows on the host
    acc0 = nc.dram_tensor("acc0", [128, M], _F32, kind="ExternalOutput")
    # jobs 1-2 per-strip row folds
    maxs = nc.dram_tensor("maxs", [2, 128, ROW_TILES], _F32, kind="ExternalOutput")

    H = M // 2
    with tile.TileContext(nc) as tc:
        with (
            tc.tile_pool(name="inp", bufs=3) as inp_pool,
            tc.tile_pool(name="psum", bufs=2, space="PSUM") as psum_pool,
            tc.tile_pool(name="acc", bufs=1) as acc_pool,
            tc.tile_pool(name="res", bufs=3) as res_pool,
            tc.tile_pool(name="upper", bufs=8) as upper_pool,
        ):
            for j in range(NJOBS):
                ops_sb = inp_pool.tile([128, 2 * N], _BF16, tag="ops")
                engines = (nc.sync, nc.scalar, nc.gpsimd) if j == 0 else (nc.sync,) * 3
                if j == 0:
                    # Priority slices: strip 0 (groups 0/32) needs only lhs
                    # cols 0:128 and its two rhs chunks per replica - load
                    # those first so the first matmuls start ~3us earlier.
                    for a, g in ((0, 0), (1, 32)):
                        eng = engines[a]
                        eng.dma_start(
                            ops_sb[g : g + KCAT, 0:128], ops[j, a][:, 0:128]
                        )
                        c0 = N + a * COL_CHUNK
                        eng.dma_start(
                            ops_sb[g : g + KCAT, c0 : c0 + COL_CHUNK],
                            ops[j, a][:, c0 : c0 + COL_CHUNK],
                        )
                        c2 = c0 + 2 * COL_CHUNK
                        eng.dma_start(
                            ops_sb[g : g + KCAT, c2 : c2 + COL_CHUNK],
                            ops[j, a][:, c2 : c2 + COL_CHUNK],
                        )
                        eng.dma_start(
                            ops_sb[g : g + KCAT, 128:N], ops[j, a][:, 128:N]
                        )
                        o = N + (1 - a) * COL_CHUNK
                        eng.dma_start(
                            ops_sb[g : g + KCAT, o : o + COL_CHUNK],
                            ops[j, a][:, o : o + COL_CHUNK],
                        )
                        o2 = o + 2 * COL_CHUNK
                        eng.dma_start(
                            ops_sb[g : g + KCAT, o2 : o2 + COL_CHUNK],
                            ops[j, a][:, o2 : o2 + COL_CHUNK],
                        )
                    engines[2].dma_start(ops_sb[64 : 64 + KCAT, :], ops[j, 2])
                else:
                    for a, g in enumerate((0, 32, 64)):
                        engines[a].dma_start(ops_sb[g : g + KCAT, :], ops[j, a])

                if j == 0:
                    # --- single-P dual-fold: rows AND column accumulation ---
                    rowm_sb = res_pool.tile([128, 2 * ROW_TILES], _F32, tag="rowm")
                    acc_a = acc_pool.tile([128, M], _F32, tag="acc_a")
                    acc_b = acc_pool.tile([128, M], _F32, tag="acc_b")
                    for i in range(ROW_TILES):
                        lo_ps = psum_pool.tile([128, H], _F32, tag="lo")
                        hi_ps = psum_pool.tile([128, H], _F32, tag="hi")
                        li = slice(i * 128, (i + 1) * 128)
                        for c in range(4):
                            # strip 0 alternates two groups so its matmuls
                            # gate on two DMA queues, not three (ramp)
                            g = (c % 2) * 32 if i == 0 else ((i * 4 + c) % 3) * 32
                            cs = slice(c * COL_CHUNK, (c + 1) * COL_CHUNK)
                            dst = lo_ps if c < 2 else hi_ps
                            ds = slice((c % 2) * COL_CHUNK, (c % 2 + 1) * COL_CHUNK)
                            nc.tensor.matmul(
                                dst[:, ds],
                                ops_sb[g : g + KCAT, li],
                                ops_sb[g : g + KCAT, N + cs.start : N + cs.stop],
                                start=True,
                                stop=True,
                            )
                        cur, prv = (acc_a, acc_b) if i % 2 == 0 else (acc_b, acc_a)
                        for h, ps in ((0, lo_ps), (1, hi_ps)):
                            hs = slice(h * H, (h + 1) * H)
                            rs = slice(2 * i + h, 2 * i + h + 1)
                            if i == 0:
                                nc.vector._custom_dve(
                                    COPY_FOLD,
                                    out=cur[:, hs],
                                    in0=ps[:],
                                    accum_out=rowm_sb[:, rs],
                                )
                            else:
                                nc.vector._custom_dve(
                                    MAXPAIR_FOLD,
                                    out=cur[:, hs],
                                    in0=ps[:],
                                    in1=prv[:, hs],
                                    accum_out=rowm_sb[:, rs],
                                )
                    final_acc = acc_a if (ROW_TILES - 1) % 2 == 0 else acc_b
                    # Column fold on host; this DMA hides under jobs 1-2.
                    # NOT on the sync queue: these waits would block jobs
                    # 1-2's input loads behind them (in-order DMA queues).
                    nc.scalar.dma_start(acc0[:, 0:H], final_acc[:, 0:H])
                    nc.gpsimd.dma_start(acc0[:, H:M], final_acc[:, H:M])
                    nc.gpsimd.dma_start(rowm[:, :], rowm_sb[:])
                else:
                    # --- two-direction paired fold (v1 body): rows only ---
                    maxs_sb = res_pool.tile([128, ROW_TILES], _F32, tag="maxs")
                    for i in range(ROW_TILES):
                        dummy = upper_pool.tile([128, 1], _F32, tag="dummy")
                        # strip 0: take the other ring's slot (freed one DVE
                        # instruction earlier) so the ACT copy chain starts
                        # sooner at the job boundary
                        t_hi, t_lo = ("lo", "hi") if i == 0 else ("hi", "lo")
                        hi_ps = psum_pool.tile([128, H], _F32, tag=t_hi)
                        lo_ps = psum_pool.tile([128, H], _F32, tag=t_lo)
                        li = slice(i * 128, (i + 1) * 128)
                        # upper-half chunks first so the ACT copy overlaps
                        # the PE filling the lower half
                        for k, (dst, half) in enumerate(
                            ((hi_ps, 0), (hi_ps, 1), (lo_ps, 0), (lo_ps, 1))
                        ):
                            g = ((i * 4 + k) % 3) * 32
                            c = 2 + k if k < 2 else k - 2
                            cs = slice(c * COL_CHUNK, (c + 1) * COL_CHUNK)
                            nc.tensor.matmul(
                                dst[:, half * COL_CHUNK : (half + 1) * COL_CHUNK],
                                ops_sb[g : g + KCAT, li],
                                ops_sb[g : g + KCAT, N + cs.start : N + cs.stop],
                                start=True,
                                stop=True,
                            )
                            if k == 1:
                                upper = upper_pool.tile([128, H], _F32, tag="upper")
                                nc.scalar.copy(upper[:], hi_ps[:])
                        nc.vector._custom_dve(
                            MAX2_REDUCE,
                            out=dummy.broadcast_to((128, H)),
                            in0=lo_ps[:],
                            in1=upper[:],
                            accum_out=maxs_sb[:, i : i + 1],
                        )
                    nc.sync.dma_start(maxs[j - 1], maxs_sb[:])
    nc.compile()
    return nc


def _get_nc():
    global _cached_nc
    if _cached_nc is None:
        _cached_nc = _build_nc()
    return _cached_nc


def _augment(a, b):
    """a: [n, 3], b: [m, 3] -> (lhsT [5, n], rhs [5, m]) float32, negated."""
    n = a.shape[0]
    m = b.shape[0]
    lhsT = np.empty((5, n), dtype=np.float32)
    lhsT[0:3] = -a.T
    lhsT[3] = -(a * a).sum(axis=1)
    lhsT[4] = -1.0
    rhs = np.empty((5, m), dtype=np.float32)
    rhs[0:3] = -2.0 * b.T
    rhs[3] = 1.0
    rhs[4] = (b * b).sum(axis=1)
    return lhsT, rhs


def _split_cat(lhs, rhs):
    """fp32 [J, 5, n] pairs -> packed K-stacked bf16 [J, 3, KCAT, 2n]."""
    lh = lhs.astype(_NP_BF16)
    ll = (lhs - lh.astype(np.float32)).astype(_NP_BF16)
    rh = rhs.astype(_NP_BF16)
    rl = (rhs - rh.astype(np.float32)).astype(_NP_BF16)
    lcat = np.concatenate([lh, lh, ll], axis=1)
    rcat = np.concatenate([rh, rl, rh], axis=1)
    packed = np.concatenate([lcat, rcat], axis=2)  # [J, KCAT, 2N]
    return np.ascontiguousarray(np.repeat(packed[:, None, :, :], 3, axis=1))


def _in_maps(predictions, targets):
    in_maps = []
    for core in range(N_CORES):
        b0, b1 = 2 * core, 2 * core + 1
        lhs = np.empty((NJOBS, 5, N), dtype=np.float32)
        rhs = np.empty((NJOBS, 5, M), dtype=np.float32)
        lhs[0], rhs[0] = _augment(predictions[b0], targets[b0])
        lhs[1], rhs[1] = _augment(predictions[b1], targets[b1])
        lhs[2], rhs[2] = _augment(targets[b1], predictions[b1])
        in_maps.append({"ops": _split_cat(lhs, rhs)})
    return in_maps


def _host_reduce(results):
    total = 0.0
    for core in range(N_CORES):
        r = results[core]
        pairs = r["rowm"].astype(np.float64).reshape(128, ROW_TILES, 2)
        total -= pairs.max(axis=-1).sum()
        total -= r["acc0"].astype(np.float64).max(axis=0).sum()
        total -= r["maxs"].astype(np.float64).sum()
    return np.float32(total)


def kernel(predictions, targets):
    predictions = np.asarray(predictions, dtype=np.float32)
    targets = np.asarray(targets, dtype=np.float32)

    nc = _get_nc()
    res = run_bass_kernel_spmd(
        nc, _in_maps(predictions, targets), core_ids=list(range(N_CORES))
    )
    return _host_reduce(res.results)
